# revision 3
# baseline (speedup 1.0000x reference)
"""Trainium2 Bass kernel v2 for nn_Depthawaregate (depth-aware gated shuffled conv).

Math (per sample):
  mx/av   = channel max/mean of fmapD                      [H,W,1] each
  d       = conv5x5([mx||av], w_sa, SAME)                  [H,W,1]
  gate_k  = exp(-2|d(y+dy,x+dx) - d(y,x)|)  (zero-pad d)   k = 3x3 taps
  gated_k = gate_k * fmapS_pad(y+dy, x+dx, :)              (zero-pad fmapS)
  conv[y, 3t+p] = sum_{ki,kj} gated_{k*}[y, t+delta] @ W[ki,kj]
      with k* = (6*ki + 3p + kj) mod 9, delta in {0,42,43,85,86}
  out     = relu(BN(conv)) + fmapS

v2 design (vs the v1 GPSIMD-gating kernel):
  - fp16 everywhere on-chip; f32 only in HBM in/out.
  - Xt16 (channel-major fmapS, fp16, zero-padded m-space) built via one
    XBAR dma_start_transpose from an fp16 DRAM scratch copy (no PE
    transposes).
  - 5x5 depth conv computed on-chip as a separable-style two-pass FMA
    (vertical taps in [x,y] planes on DVE/GPSIMD, 5 PE transposes,
    horizontal adds) -- no DRAM im2col roundtrip.
  - Gate planes flattened to an m-space DRAM row per tap (gateM); per
    strip the 8 gated operand tensors are produced by 3 SBUF->SBUF
    window-copy DMAs from Xt16 plus ONE gpsimd accum_op=mult DMA that
    broadcasts gateM across partitions (stride-0 DRAM source) and
    multiplies in place. Gating costs no DVE/ACT/GPSIMD compute.
  - Conv: per strip-pair, (ki,kj)-outer loop so each fp16 weight load
    serves 6 PSUM-accumulated matmuls (2 strips x 3 phases).
  - BN+ReLU on ACT straight into a phase-interleaved fp16 row buffer;
    XBAR dma transpose to pixel-major; residual added from an fp16 copy
    of fmapS (gpsimd cast DMA); output written by a gpsimd cast DMA
    (fp16 -> f32), one DMA per strip.

Sharding: pure data parallel over batch B=8 -> 8 cores, one sample each.
"""

import sys

sys.path.insert(0, "/opt/trn_rl_repo")
sys.path.insert(0, "/root/.axon_site/_ro/trn_rl_repo")

import numpy as np
from contextlib import ExitStack

import concourse.bass as bass
import concourse.tile as tile
import concourse.mybir as mybir
from concourse import bacc

dt = mybir.dt
AF = mybir.ActivationFunctionType
ALU = mybir.AluOpType

H = W = 128
C = 128
CD = 64
EPS = 1e-3
A_GATE = -2.0

M0 = 256                     # m index of pixel (0,0); 512B-aligned for XBAR
XT_LEN = M0 + 16384 + 192    # lead pad + pixels + tail pad (taps read +-129)

TAPS = [(dy, dx) for dy in (-1, 0, 1) for dx in (-1, 0, 1)]  # k = 3*(dy+1)+(dx+1)
CENTER_K = 4
TAPK = [0, 1, 2, 3, 5, 6, 7, 8]
IDX = {k: i for i, k in enumerate(TAPK)}  # gr/gateM row order

T_P = [43, 43, 42]
STRIPS = [(11 * s, 11) for s in range(11)] + [(121, 7)]

# paired-tap gating: (first m-offset rel m0c, offset step) for idx pairs
# idx order offsets: -129,-128,-127,-1,+1,+127,+128,+129
PAIR_OFFS = [(-129, 1), (-127, 126), (1, 126), (128, 1)]


def pass_params(ki, p, kj):
    v = 6 * ki + 3 * p + kj
    ks = v % 9
    delta = (384 * ki + 3 * p + kj - ks) // 9
    return ks, delta


def body(tc, out_d, ins, dbg=None):
    nc = tc.nc
    fD = ins["fmapD"]      # [H*W, CD] f32 (flat pixel-major)
    fS = ins["fmapS"]      # [H*W, C] f32
    w50_d = ins["w50"]     # [128, 50] f32 (replicated across partitions)
    w_cv = ins["w_conv"]   # [9, C, C] fp16
    bns_d = ins["bn_s"]    # [C] f32
    bnb_d = ins["bn_b"]    # [C] f32
    id_d = ins["ident"]    # [128, 128] f32

    with ExitStack() as ctx:
        P = ctx.enter_context(tc.tile_pool(name="persist", bufs=1))
        dram = ctx.enter_context(tc.tile_pool(name="dram", bufs=1, space="DRAM"))

        ident = P.tile([128, 128], dt.float32, tag="ident")
        nc.sync.dma_start(ident[:], id_d[:])
        w50 = P.tile([128, 50], dt.float32, tag="w50")
        nc.sync.dma_start(w50[:], w50_d[:])
        wc = P.tile([128, 9 * C], dt.float16, tag="wconv")
        # w_conv [k, c, o] -> wc [c, (k, o)]
        nc.sync.dma_start(
            wc[:],
            bass.AP(w_cv.tensor, w_cv.offset, [[128, 128], [C * C, 9], [1, 128]]),
        )
        bns = P.tile([128, 1], dt.float32, tag="bns")
        nc.sync.dma_start(bns[:], bns_d[:])
        bnb = P.tile([128, 1], dt.float32, tag="bnb")
        nc.sync.dma_start(bnb[:], bnb_d[:])

        fS16 = dram.tile([H * W, C], dt.float16, tag="fS16")
        gateM = dram.tile([8, XT_LEN], dt.float16, tag="gateM")

        if dbg is not None:
            nc.sync.dma_start(dbg["xt"][:], Xt16[:])

        # ---- Stage A: channel max / mean of fmapD -> [x, y] planes ----
        mxP = P.tile([128, 132], dt.float32, tag="mxP")  # [x, 2+y+2]
        avP = P.tile([128, 132], dt.float32, tag="avP")
        nc.vector.memset(mxP[:], 0.0)
        nc.vector.memset(avP[:], 0.0)
        with tc.tile_pool(name="fD", bufs=2) as fDp:
            for i in range(8):
                t = fDp.tile([128, 16, CD], dt.float32, tag="fD")
                src = bass.AP(
                    fD.tensor,
                    fD.offset + i * 16 * W * CD,
                    [[CD, 128], [W * CD, 16], [1, CD]],
                )
                nc.sync.dma_start(t[:], src)
                nc.vector.tensor_reduce(
                    mxP[:, 2 + 16 * i : 18 + 16 * i], t[:],
                    axis=mybir.AxisListType.X, op=ALU.max,
                )
                nc.vector.tensor_reduce(
                    avP[:, 2 + 16 * i : 18 + 16 * i], t[:],
                    axis=mybir.AxisListType.X, op=ALU.add,
                )

        # ---- Stage B: 5x5 conv on [mx||av], separable two-pass ----
        # pass 1 (vertical, free dim y in [x,y] planes):
        #   t_kj[x, y] = sum_{ch, ki} w[ch,ki,kj] * plane_ch[x, y+ki-2]
        # w50 column order: j = kj*10 + ki*2 + ch
        dC = P.tile([128, 132], dt.float32, tag="dC")   # [y, 2+x+2]
        d3m = P.tile([128, 132], dt.float32, tag="d3m")  # rows y-1
        d3p = P.tile([128, 132], dt.float32, tag="d3p")  # rows y+1
        nc.vector.memset(dC[:], 0.0)
        nc.vector.memset(d3m[:], 0.0)
        nc.vector.memset(d3p[:], 0.0)
        with tc.tile_pool(name="tk", bufs=1) as tkp, tc.tile_pool(
            name="acc", bufs=2
        ) as accp, tc.tile_pool(name="psB", bufs=2, space="PSUM") as psB, tc.tile_pool(
            name="tT", bufs=1
        ) as tTp:
            tTs = []
            for kj in range(5):
                eng = nc.vector
                cur = accp.tile([128, 128], dt.float32, tag=f"acc{kj}", name=f"acc{kj}_first")
                eng.tensor_scalar_mul(cur[:], mxP[:, 0:128], w50[:, 10 * kj : 10 * kj + 1])
                terms = [(ki, ch) for ki in range(5) for ch in range(2)][1:]
                for n, (ki, ch) in enumerate(terms):
                    plane = mxP if ch == 0 else avP
                    j = 10 * kj + 2 * ki + ch
                    if n == len(terms) - 1:
                        nxt = tkp.tile(
                            [128, 128], dt.float32, tag=f"tk{kj}", name=f"tk{kj}"
                        )
                    else:
                        nxt = accp.tile(
                            [128, 128], dt.float32, tag=f"acc{kj}",
                            name=f"acc{kj}_{n}",
                        )
                    eng.scalar_tensor_tensor(
                        out=nxt[:], in0=plane[:, ki : ki + 128],
                        scalar=w50[:, j : j + 1], in1=cur[:],
                        op0=ALU.mult, op1=ALU.add,
                    )
                    cur = nxt
                # transpose [x, y] -> [y, x]
                pt = psB.tile([128, 128], dt.float32, tag="psB")
                nc.tensor.transpose(pt[:], cur[:], ident[:])
                tT = tTp.tile([128, 132], dt.float32, tag=f"tT{kj}")
                nc.vector.memset(tT[:], 0.0)
                nc.scalar.activation(tT[:, 2:130], pt[:], AF.Copy)
                tTs.append(tT)
            # pass 2 (horizontal adds): d[y, x] = sum_kj tT_kj[y, x+kj-2]
            a = accp.tile([128, 128], dt.float32, tag="pa0")
            nc.vector.tensor_add(a[:], tTs[0][:, 0:128], tTs[1][:, 1:129])
            b = accp.tile([128, 128], dt.float32, tag="pa1")
            nc.vector.tensor_add(b[:], a[:], tTs[2][:, 2:130])
            a2 = accp.tile([128, 128], dt.float32, tag="pa2")
            nc.vector.tensor_add(a2[:], b[:], tTs[3][:, 3:131])
            nc.vector.tensor_add(dC[:, 2:130], a2[:], tTs[4][:, 4:132])

        # y-shifted copies for the dy=+-1 taps
        nc.gpsimd.dma_start(d3m[1:128, :], dC[0:127, :])
        nc.gpsimd.dma_start(d3p[0:127, :], dC[1:128, :])

        # ---- Stage C: gates -> gateM rows in DRAM (m-space) ----
        with tc.tile_pool(name="gu", bufs=2) as gup, tc.tile_pool(
            name="ga", bufs=2
        ) as gap, tc.tile_pool(name="gi", bufs=3) as gip:
            for k in TAPK:
                dy, dx = TAPS[k]
                dN = {-1: d3m, 0: dC, 1: d3p}[dy]
                u = gup.tile([128, 128], dt.float32, tag="gu")
                nc.vector.tensor_sub(u[:], dN[:, 2 + dx : 130 + dx], dC[:, 2:130])
                au = gap.tile([128, 128], dt.float32, tag="gau")
                nc.scalar.activation(au[:], u[:], AF.Abs)
                gi = gip.tile([128, 128], dt.float16, tag="gi")
                nc.scalar.activation(gi[:], au[:], AF.Exp, scale=A_GATE)
                if dx == -1:
                    nc.vector.memset(gi[:, 0:1], 0.0)
                if dx == 1:
                    nc.vector.memset(gi[:, 127:128], 0.0)
                gma = gateM[:]
                dst = bass.AP(
                    gma.tensor, gma.offset + IDX[k] * XT_LEN + M0, [[1, H * W]]
                )
                nc.gpsimd.dma_start(dst, gi[:])

        if dbg is not None:
            gmbuf = P.tile([8, XT_LEN], dt.float16, tag="gmdbg")
            nc.sync.dma_start(gmbuf[:], gateM[:])
            nc.sync.dma_start(dbg["gm"][:], gmbuf[:])
            nc.sync.dma_start(dbg["d"][:], dC[:])
        # ---- Xt16: fp16 cast roundtrip + XBAR transpose ----
        Xt16 = P.tile([128, XT_LEN], dt.float16, tag="Xt16")
        nc.gpsimd.memset(Xt16[:, 0:M0], 0.0)
        nc.gpsimd.memset(Xt16[:, M0 + 16384 : XT_LEN], 0.0)
        with tc.tile_pool(name="xs", bufs=2) as xsp, tc.tile_pool(
            name="xs16", bufs=2
        ) as x16p:
            for i in range(8):
                xs = xsp.tile([128, 16, 128], dt.float32, tag="xs")
                src = bass.AP(
                    fS.tensor,
                    fS.offset + i * 2048 * 128,
                    [[128, 128], [128 * 128, 16], [1, 128]],
                )
                nc.sync.dma_start(xs[:], src)
                x16 = x16p.tile([128, 16, 128], dt.float16, tag="x16")
                nc.scalar.activation(x16[:], xs[:], AF.Copy)
                f16a = fS16[:]
                dst = bass.AP(
                    f16a.tensor,
                    f16a.offset + i * 2048 * 128,
                    [[128, 128], [128 * 128, 16], [1, 128]],
                )
                nc.scalar.dma_start(dst, x16[:])
        for ch in range(4):
            eng = nc.sync if ch % 2 == 0 else nc.scalar
            eng.dma_start_transpose(
                Xt16[:, M0 + 4096 * ch : M0 + 4096 * (ch + 1)],
                fS16[4096 * ch : 4096 * (ch + 1), :],
            )
        # ---- Stage E: gated shuffled conv, per strip ----
        xta = Xt16[:]
        gma = gateM[:]
        with tc.tile_pool(name="gr", bufs=4) as grp, tc.tile_pool(
            name="grep", bufs=2
        ) as grepp, tc.tile_pool(
            name="psE", bufs=2, space="PSUM"
        ) as psE, tc.tile_pool(name="bnr", bufs=2) as bnrp, tc.tile_pool(
            name="ot16", bufs=2
        ) as otp, tc.tile_pool(name="otf", bufs=2) as otfp, tc.tile_pool(
            name="fsr", bufs=2
        ) as fsrp:
            for u in range(6):
                pair = STRIPS[2 * u : 2 * u + 2]
                grs = {}
                for y0, nrows in pair:
                    m0c = M0 + 128 * y0
                    mt = nrows * 128
                    grep_t = grepp.tile([128, 8, 11 * 128], dt.float16, tag="grep")
                    rep = bass.AP(
                        gma.tensor, gma.offset + m0c, [[0, 128], [XT_LEN, 8], [1, mt]]
                    )
                    nc.sync.dma_start(grep_t[:, :, 0:mt], rep)
                    gr = grp.tile([128, 8, 11 * 128], dt.float16, tag="gr")
                    for pi, (off0, step) in enumerate(PAIR_OFFS):
                        in0 = bass.AP(
                            xta.tensor,
                            xta.offset + m0c + off0,
                            [xta.ap[0], [step, 2], [1, mt]],
                        )
                        nc.vector.tensor_mul(
                            gr[:, 2 * pi : 2 * pi + 2, 0:mt],
                            in0,
                            grep_t[:, 2 * pi : 2 * pi + 2, 0:mt],
                        )
                    grs[y0] = gr
                    if dbg is not None and y0 == 0:
                        nc.sync.dma_start(dbg["rep"][:], grep_t[:])
                        nc.sync.dma_start(dbg["gr"][:], gr[:])
                psss = {}
                for y0, nrows in pair:
                    psss[y0] = [
                        psE.tile([128, nrows * T_P[p]], dt.float32,
                                 tag=f"psE{p}", name=f"psE{y0}_{p}")
                        for p in range(3)
                    ]
                for y0, nrows in pair:
                    m0c = M0 + 128 * y0
                    for ki in range(3):
                        for kj in range(3):
                            wsl = wc[:, (3 * ki + kj) * C : (3 * ki + kj + 1) * C]
                            for p in range(3):
                                ks, delta = pass_params(ki, p, kj)
                                if ks == CENTER_K:
                                    base, off = xta, xta.offset + m0c + delta
                                else:
                                    ga = grs[y0][:]
                                    base, off = ga, (
                                        ga.offset + IDX[ks] * 11 * 128 + delta
                                    )
                                rhs = bass.AP(
                                    base.tensor, off,
                                    [base.ap[0] if base is not xta else xta.ap[0],
                                     [128, nrows], [1, T_P[p]]],
                                )
                                nc.tensor.matmul(
                                    psss[y0][p][:], wsl, rhs,
                                    start=(ki == 0 and kj == 0),
                                    stop=(ki == 2 and kj == 2),
                                )  # per-strip loop
                for y0, nrows in pair:
                    mt = nrows * 128
                    bnr = bnrp.tile([128, 11 * 128], dt.float16, tag="bnr")
                    ba = bnr[:]
                    for p in range(3):
                        dst = bass.AP(
                            ba.tensor, ba.offset + p,
                            [ba.ap[0], [128, nrows], [3, T_P[p]]],
                        )
                        nc.scalar.activation(
                            dst, psss[y0][p][:], AF.Relu,
                            bias=bnb[:, 0:1], scale=bns[:, 0:1],
                        )
                    ot16 = otp.tile([128, 11, 128], dt.float16, tag="ot16")
                    nc.scalar.dma_start_transpose(
                        ot16[:, 0:nrows, :], bnr[:, 0:mt]
                    )
                    fsr = fsrp.tile([128, 11, 128], dt.float16, tag="fsr")
                    src = bass.AP(
                        fS.tensor,
                        fS.offset + y0 * 128 * 128,
                        [[128, 128], [128 * 128, nrows], [1, 128]],
                    )
                    nc.gpsimd.dma_start(fsr[:, 0:nrows, :], src)
                    otf = otfp.tile([128, 11, 128], dt.float16, tag="otf")
                    nc.vector.tensor_add(
                        otf[:, 0:nrows, :], ot16[:, 0:nrows, :], fsr[:, 0:nrows, :]
                    )
                    dst = bass.AP(
                        out_d.tensor,
                        out_d.offset + y0 * 128 * 128,
                        [[128, 128], [128 * 128, nrows], [1, 128]],
                    )
                    nc.gpsimd.dma_start(dst, otf[:, 0:nrows, :])


def build():
    nc = bacc.Bacc("TRN2", target_bir_lowering=False, debug=False)

    ins = dict(
        fmapD=nc.dram_tensor("fmapD", [H * W, CD], dt.float32, kind="ExternalInput").ap(),
        fmapS=nc.dram_tensor("fmapS", [H * W, C], dt.float32, kind="ExternalInput").ap(),
        w50=nc.dram_tensor("w50", [128, 50], dt.float32, kind="ExternalInput").ap(),
        w_conv=nc.dram_tensor("w_conv", [9, C, C], dt.float16, kind="ExternalInput").ap(),
        bn_s=nc.dram_tensor("bn_s", [C], dt.float32, kind="ExternalInput").ap(),
        bn_b=nc.dram_tensor("bn_b", [C], dt.float32, kind="ExternalInput").ap(),
        ident=nc.dram_tensor("ident", [128, 128], dt.float32, kind="ExternalInput").ap(),
    )
    out_d = nc.dram_tensor("out", [H, W, C], dt.float32, kind="ExternalOutput").ap()

    with tile.TileContext(nc) as tc:
        body(tc, out_d, ins)

    nc.compile()
    return nc


def prep_inputs(inputs):
    """Host-side prep; returns per-core input maps."""
    w_sa = np.asarray(inputs["w_sa"], np.float32)  # [ki, kj, ch, 1]
    w = w_sa[:, :, :, 0].copy()                    # [ki, kj, ch]
    w[:, :, 1] /= CD                               # fold channel-mean divisor
    # w50 col j = kj*10 + ki*2 + ch
    w50 = np.zeros((50,), np.float32)
    for kj in range(5):
        for ki in range(5):
            for ch in range(2):
                w50[kj * 10 + ki * 2 + ch] = w[ki, kj, ch]
    w50rep = np.ascontiguousarray(np.broadcast_to(w50, (128, 50))).astype(np.float32)

    w_conv = np.asarray(inputs["w_conv"], np.float32).reshape(9, C, C).astype(np.float16)

    gamma = np.asarray(inputs["gamma"], np.float64)
    beta = np.asarray(inputs["beta"], np.float64)
    mm = np.asarray(inputs["mov_mean"], np.float64)
    mv = np.asarray(inputs["mov_var"], np.float64)
    s = gamma / np.sqrt(mv + EPS)
    b = beta - mm * s
    bn_s = s.astype(np.float32)
    bn_b = b.astype(np.float32)

    ident = np.eye(128, dtype=np.float32)

    fmapD = np.asarray(inputs["fmapD"], np.float32)
    fmapS = np.asarray(inputs["fmapS"], np.float32)
    in_maps = []
    for i in range(8):
        in_maps.append(
            dict(
                fmapD=np.ascontiguousarray(fmapD[i].reshape(H * W, CD)),
                fmapS=np.ascontiguousarray(fmapS[i].reshape(H * W, C)),
                w50=w50rep,
                w_conv=w_conv,
                bn_s=bn_s,
                bn_b=bn_b,
                ident=ident,
            )
        )
    return in_maps


_NC = None
LAST_EXEC_NS = None


def get_nc():
    global _NC
    if _NC is None:
        _NC = build()
    return _NC


def run(inputs, trace=False):
    global LAST_EXEC_NS
    from concourse.bass_utils import run_bass_kernel_spmd

    nc = get_nc()
    in_maps = prep_inputs(inputs)
    r = run_bass_kernel_spmd(nc, in_maps, list(range(8)), trace=trace)
    if r.exec_time_ns is not None:
        LAST_EXEC_NS = r.exec_time_ns
    out = np.stack([r.results[i]["out"] for i in range(8)], axis=0)
    return out


def kernel(**inputs) -> np.ndarray:
    return run(inputs, trace=False)
